# revision 6
# baseline (speedup 1.0000x reference)
"""Trainium2 Bass kernel for nn_CYPClassifier (vq_codebook).

Data-parallel over batch B=64 across 8 NeuronCores (8 rows/core).
Heavy compute (two 1280->768 k=9 conv1ds as 9-tap shifted matmuls in fp16,
attention pooling, projections, VQ distance/argmin/histogram, evidence head)
runs on device; the host only shards inputs, folds BN constants into
weights, sums per-core partial losses/histograms and evaluates the final
closed-form scalars (digamma/gammaln on 64x2 logits).
"""

import sys

import numpy as np

for _p in ("/opt/trn_rl_repo", "/root/.axon_site/_ro/trn_rl_repo"):
    if _p not in sys.path:
        sys.path.append(_p)

import concourse.bacc as bacc
import concourse.bass as bass
import concourse.tile as tile
from concourse import mybir
from concourse.bass_utils import run_bass_kernel_spmd

F32 = mybir.dt.float32
F16 = mybir.dt.float16
ALU = mybir.AluOpType
ACT = mybir.ActivationFunctionType
AX = mybir.AxisListType

NCORES = 8
B = 64
BL = B // NCORES          # batch rows per core
L = 1024
LP = L + 8                # padded length (4 each side)
CI, CO = 1280, 768        # conv in/out channels
NCI, NCO = CI // 128, CO // 128
KW = 9
KCB, CD = 512, 64         # codebook size, code dim
NS = BL * 16              # VQ slices per core (z is 1024 = 16*64 wide)
BN_EPS = 1e-5
EVD_LAMBDA = 0.1
N_CLASS = 2


def _group_ranges(bl):
    out = []
    s = 0
    while s < bl:
        n = min(4, bl - s)
        out.append((s, n))
        s += n
    return out


def build_nc(bl=BL):
    ns = bl * 16
    nc = bacc.Bacc("TRN2", target_bir_lowering=False, debug=False)

    # ---------------- DRAM I/O ----------------
    xt = nc.dram_tensor("xt", [bl, NCI, 128, LP], F16, kind="ExternalInput")
    wslab = nc.dram_tensor("wslab", [2, NCO, NCI, 128, KW, 128], F16, kind="ExternalInput")
    madd = nc.dram_tensor("madd", [bl, L], F32, kind="ExternalInput")
    dembT = nc.dram_tensor("dembT", [6, 128, bl], F32, kind="ExternalInput")
    wd = nc.dram_tensor("wd", [6, 128, 8, 128], F32, kind="ExternalInput")
    bd = nc.dram_tensor("bd", [8, 128], F32, kind="ExternalInput")
    battn = nc.dram_tensor("battn", [NCO, 128], F32, kind="ExternalInput")
    bfeat = nc.dram_tensor("bfeat", [NCO, 128], F32, kind="ExternalInput")
    wp = nc.dram_tensor("wp", [12, 128, 8, 128], F32, kind="ExternalInput")
    bp = nc.dram_tensor("bp", [8, 128], F32, kind="ExternalInput")
    cbaug = nc.dram_tensor("cbaug", [2, 65, KCB], F32, kind="ExternalInput")
    cbt = nc.dram_tensor("cbt", [2, 4, 128, CD], F32, kind="ExternalInput")
    eye = nc.dram_tensor("eye", [128, 128], F32, kind="ExternalInput")
    wh = nc.dram_tensor("wh", [16, 128, 4, 128], F32, kind="ExternalInput")
    bh = nc.dram_tensor("bh", [4, 128], F32, kind="ExternalInput")
    wh2 = nc.dram_tensor("wh2", [4, 128, 2], F32, kind="ExternalInput")
    bh2 = nc.dram_tensor("bh2", [2, 1], F32, kind="ExternalInput")

    out_counts_d = nc.dram_tensor("out_counts_d", [1, KCB], F32, kind="ExternalOutput")
    out_counts_t = nc.dram_tensor("out_counts_t", [1, KCB], F32, kind="ExternalOutput")
    out_scalars = nc.dram_tensor("out_scalars", [1, 8], F32, kind="ExternalOutput")
    out_logits = nc.dram_tensor("out_logits", [2, bl], F32, kind="ExternalOutput")

    groups = _group_ranges(bl)
    gmax_n = max(n for _, n in groups)

    with tile.TileContext(nc) as tc:
        with (
            tc.tile_pool(name="const", bufs=1) as cpool,
            tc.tile_pool(name="persist", bufs=1) as ppool,
            tc.tile_pool(name="xp", bufs=1) as xpool,
            tc.tile_pool(name="slab", bufs=4) as slabpool,
            tc.tile_pool(name="maddp", bufs=1) as maddpool,
            tc.tile_pool(name="work", bufs=3) as workpool,
            tc.tile_pool(name="exp", bufs=1) as expool,
            tc.tile_pool(name="scr", bufs=3) as scrpool,
            tc.tile_pool(name="small", bufs=4) as smpool,
            tc.tile_pool(name="wt", bufs=6) as wpool,
            tc.tile_pool(name="vq", bufs=2) as vqpool,
            tc.tile_pool(name="ps", bufs=8, space="PSUM") as pspool,
        ):
            # ---------------- constants ----------------
            ones = cpool.tile([128, 1], F32)
            nc.vector.memset(ones[:], 1.0)
            eye_sb = cpool.tile([128, 128], F32)
            nc.sync.dma_start(eye_sb[:], eye[:])

            def col_tile(dram, ncols):
                t = cpool.tile([128, ncols], F32, tag=dram.name + "_sb")
                nc.sync.dma_start(t[:], dram[:].rearrange("m p -> p m"))
                return t

            bd_sb = col_tile(bd, 8)
            battn_sb = col_tile(battn, NCO)
            bfeat_sb = col_tile(bfeat, NCO)
            bp_sb = col_tile(bp, 8)
            bh_sb = col_tile(bh, 4)
            bh2_sb = cpool.tile([2, 1], F32)
            nc.sync.dma_start(bh2_sb[:], bh2[:])

            dembT_sb = cpool.tile([128, 6, bl], F32)
            nc.sync.dma_start(dembT_sb[:], dembT[:].rearrange("k p b -> p k b"))

            def leaky_from_psum(out_ap, ps_ap, bias_ap, tmp_shape, tag):
                tmp = smpool.tile(tmp_shape, F32, tag=tag, name=tag)
                nc.scalar.activation(tmp[:], ps_ap, ACT.Identity, bias=bias_ap)
                nc.vector.scalar_tensor_tensor(out_ap, in0=tmp[:], scalar=0.01,
                                               in1=tmp[:], op0=ALU.mult, op1=ALU.max)

            # ---------------- drug encoder ----------------
            # d_zT[c, b] = leaky(sum_f Wd[f, c] * d_emb[b, f] + bd[c])
            dzT = ppool.tile([128, 8, bl], F32)
            for m in range(8):
                ps = pspool.tile([128, 512], F32, tag="ps")
                for kt in range(6):
                    wtile = wpool.tile([128, 128], F32, tag="wt")
                    nc.sync.dma_start(wtile[:], wd[kt, :, m, :])
                    nc.tensor.matmul(ps[:, 0:bl], wtile[:], dembT_sb[:, kt, :],
                                     start=(kt == 0), stop=(kt == 5))
                leaky_from_psum(dzT[:, m, :], ps[:, 0:bl], bd_sb[:, m:m + 1],
                                [128, bl], "lk_d")

            # ---------------- target conv encoder ----------------
            combT = ppool.tile([128, 12, bl], F32)  # [p, ktile, b]: 0-5 weighted, 6-11 gmax

            for g0, gn in groups:
                xs = {}
                madd_sb = {}
                for bi in range(gn):
                    b = g0 + bi
                    for ci in range(NCI):
                        t = xpool.tile([128, LP], F16, tag=f"x_{bi}_{ci}", name=f"x_{bi}_{ci}")
                        nc.sync.dma_start(t[:], xt[b, ci])
                        xs[bi, ci] = t
                    mt = maddpool.tile([128, L], F32, tag=f"madd_{bi}", name=f"madd_{bi}")
                    src = madd[b, :]
                    nc.sync.dma_start(
                        mt[:],
                        bass.AP(tensor=src.tensor, offset=src.offset,
                                ap=[[0, 128]] + list(src.ap)),
                    )
                    madd_sb[bi] = mt

                for co in range(NCO):
                    exs = {}
                    rsums = {}
                    for conv_idx in (0, 1):  # 0 = attn, 1 = feat
                        pst = {}
                        for bi in range(gn):
                            for li in range(2):
                                pst[bi, li] = pspool.tile([128, 512], F32, tag="ps", name=f"pst_{bi}_{li}")
                        for ci in range(NCI):
                            slab = slabpool.tile([128, KW, 128], F16, tag="slab")
                            nc.sync.dma_start(slab[:], wslab[conv_idx, co, ci])
                            for k in range(KW):
                                lhsT = slab[:, k, :]
                                for bi in range(gn):
                                    for li in range(2):
                                        rhs = xs[bi, ci][:, li * 512 + k: li * 512 + k + 512]
                                        nc.tensor.matmul(
                                            pst[bi, li][:], lhsT, rhs,
                                            start=(ci == 0 and k == 0),
                                            stop=(ci == NCI - 1 and k == KW - 1))
                        if conv_idx == 0:
                            # attn: softmax over L with additive mask
                            for bi in range(gn):
                                am = workpool.tile([128, L], F32, tag="Am")
                                for li in range(2):
                                    nc.vector.scalar_tensor_tensor(
                                        am[:, li * 512:(li + 1) * 512],
                                        in0=pst[bi, li][:],
                                        scalar=battn_sb[:, co:co + 1],
                                        in1=madd_sb[bi][:, li * 512:(li + 1) * 512],
                                        op0=ALU.add, op1=ALU.add)
                                rmax = smpool.tile([128, 1], F32, tag="rmax")
                                nc.vector.tensor_reduce(rmax[:], am[:], axis=AX.X, op=ALU.max)
                                nrmax = smpool.tile([128, 1], F32, tag="nrmax")
                                nc.vector.tensor_scalar_mul(nrmax[:], rmax[:], -1.0)
                                ex = expool.tile([128, L], F32, tag=f"ex_{bi}")
                                sumex = smpool.tile([128, 1], F32, tag="sumex")
                                nc.scalar.activation(ex[:], am[:], ACT.Exp,
                                                     bias=nrmax[:], scale=1.0,
                                                     accum_out=sumex[:])
                                rsum = smpool.tile([128, 1], F32, tag=f"rsum_{bi}")
                                nc.vector.reciprocal(rsum[:], sumex[:])
                                exs[bi] = ex
                                rsums[bi] = rsum
                        else:
                            # feat: weighted sum with softmax weights + masked gmax
                            for bi in range(gn):
                                b = g0 + bi
                                ws0 = smpool.tile([128, 1], F32, tag="ws0")
                                ws1 = smpool.tile([128, 1], F32, tag="ws1")
                                for li, wsx in ((0, ws0), (1, ws1)):
                                    sc = scrpool.tile([128, 512], F32, tag="sc")
                                    nc.vector.scalar_tensor_tensor(
                                        sc[:],
                                        in0=pst[bi, li][:],
                                        scalar=bfeat_sb[:, co:co + 1],
                                        in1=exs[bi][:, li * 512:(li + 1) * 512],
                                        op0=ALU.add, op1=ALU.mult,
                                        accum_out=wsx[:])
                                wsum = smpool.tile([128, 1], F32, tag="wsum")
                                nc.vector.tensor_add(wsum[:], ws0[:], ws1[:])
                                nc.vector.tensor_scalar_mul(
                                    combT[:, co, bi + g0:bi + g0 + 1], wsum[:], rsums[bi][:])
                                gm0 = smpool.tile([128, 1], F32, tag="gm0")
                                gm1 = smpool.tile([128, 1], F32, tag="gm1")
                                for li, gmx in ((0, gm0), (1, gm1)):
                                    sc2 = scrpool.tile([128, 512], F32, tag="sc2", name="sc2")
                                    nc.vector.tensor_add(
                                        sc2[:], pst[bi, li][:],
                                        madd_sb[bi][:, li * 512:(li + 1) * 512])
                                    nc.vector.tensor_reduce(gmx[:], sc2[:], axis=AX.X, op=ALU.max)
                                gmc = smpool.tile([128, 1], F32, tag="gmc")
                                nc.vector.tensor_max(gmc[:], gm0[:], gm1[:])
                                nc.scalar.activation(
                                    combT[:, 6 + co, b:b + 1], gmc[:], ACT.Identity,
                                    bias=bfeat_sb[:, co:co + 1])

            # ---------------- proj ----------------
            tzT = ppool.tile([128, 8, bl], F32)
            for m in range(8):
                ps = pspool.tile([128, 512], F32, tag="ps")
                for kt in range(12):
                    wtile = wpool.tile([128, 128], F32, tag="wt")
                    nc.sync.dma_start(wtile[:], wp[kt, :, m, :])
                    nc.tensor.matmul(ps[:, 0:bl], wtile[:], combT[:, kt, :],
                                     start=(kt == 0), stop=(kt == 11))
                leaky_from_psum(tzT[:, m, :], ps[:, 0:bl], bp_sb[:, m:m + 1],
                                [128, bl], "lk_p")

            # ---------------- VQ ----------------
            def run_vq(zT, cb_idx, out_counts, scal_off, qT):
                # z_aug: [65, ns]; column n = b*16 + s holds slice s of row b
                zaug = vqpool.tile([65, ns], F32, tag="zaug")
                nc.vector.memset(zaug[64:65, :], 1.0)
                zr = zaug[0:64, :].rearrange("p (b s) -> p s b", s=16)
                for m in range(8):
                    nc.vector.tensor_copy(zr[:, 2 * m, :], zT[0:64, m, :])
                    # partitions 64-127 -> 0-63 requires a DMA
                    nc.sync.dma_start(zr[:, 2 * m + 1, :], zT[64:128, m, :])

                cba = vqpool.tile([65, KCB], F32, tag="cba")
                nc.sync.dma_start(cba[:], cbaug[cb_idx])
                eps = pspool.tile([128, 512], F32, tag="ps")
                nc.tensor.matmul(eps[0:ns, :], zaug[:], cba[:], start=True, stop=True)
                e_sb = vqpool.tile([ns, KCB], F32, tag="esb")
                nc.vector.tensor_copy(e_sb[:], eps[0:ns, :])
                mmin = smpool.tile([ns, 1], F32, tag="mmin")
                nc.vector.tensor_reduce(mmin[:], e_sb[:], axis=AX.X, op=ALU.min)
                onehot = vqpool.tile([ns, KCB], F32, tag="onehot")
                nc.vector.tensor_single_scalar(onehot[:], e_sb[:], mmin[:], ALU.is_equal)

                # histogram: counts = ones^T @ onehot
                cps = pspool.tile([128, 512], F32, tag="ps")
                nc.tensor.matmul(cps[0:1, :], ones[0:ns, :], onehot[:], start=True, stop=True)
                counts_sb = vqpool.tile([1, KCB], F32, tag="counts_sb")
                nc.vector.tensor_copy(counts_sb[:], cps[0:1, :])
                nc.sync.dma_start(out_counts[:], counts_sb[:])

                # vq loss partials: sum(|z|^2) and sum(min e)
                sps = pspool.tile([128, 512], F32, tag="ps")
                nc.tensor.matmul(sps[0:1, 0:1], mmin[:], ones[0:ns, :], start=True, stop=True)
                nc.scalar.copy(scal_sb[:, scal_off + 1:scal_off + 2], sps[0:1, 0:1])
                zsq = vqpool.tile([64, ns], F32, tag="zsq")
                nc.vector.tensor_mul(zsq[:], zaug[0:64, :], zaug[0:64, :])
                zrow = smpool.tile([64, 1], F32, tag="zrow")
                nc.vector.reduce_sum(zrow[:], zsq[:], axis=AX.X)
                z2ps = pspool.tile([128, 512], F32, tag="ps")
                nc.tensor.matmul(z2ps[0:1, 0:1], zrow[:], ones[0:64, :], start=True, stop=True)
                nc.scalar.copy(scal_sb[:, scal_off:scal_off + 1], z2ps[0:1, 0:1])

                # q = onehot @ cb via PE transpose of onehot
                qps = pspool.tile([128, 512], F32, tag="ps")
                for j in range(4):
                    tps = pspool.tile([128, 512], F32, tag="ps")
                    nc.tensor.transpose(tps[0:128, 0:ns], onehot[:, j * 128:(j + 1) * 128],
                                        eye_sb[0:ns, 0:ns])
                    ohT = vqpool.tile([128, ns], F32, tag="ohT")
                    nc.vector.tensor_copy(ohT[:], tps[0:128, 0:ns])
                    cb_sb = wpool.tile([128, CD], F32, tag="cbsb")
                    nc.sync.dma_start(cb_sb[:], cbt[cb_idx, j])
                    nc.tensor.matmul(qps[0:64, 0:ns], cb_sb[:], ohT[:],
                                     start=(j == 0), stop=(j == 3))
                nc.vector.tensor_copy(qT[:], qps[0:64, 0:ns])

            scal_sb = ppool.tile([1, 8], F32)
            nc.vector.memset(scal_sb[:], 0.0)
            qTd = ppool.tile([64, ns], F32)
            qTt = ppool.tile([64, ns], F32)
            run_vq(dzT, 0, out_counts_d, 0, qTd)
            run_vq(tzT, 1, out_counts_t, 2, qTt)
            nc.sync.dma_start(out_scalars[:], scal_sb[:])

            # ---------------- evidence head ----------------
            hT = ppool.tile([128, 16, bl], F32)
            for src_q, koff in ((qTd, 0), (qTt, 8)):
                qr = src_q[:].rearrange("p (b s) -> p s b", s=16)
                for k in range(8):
                    nc.vector.tensor_copy(hT[0:64, koff + k, :], qr[:, 2 * k, :])
                    nc.sync.dma_start(hT[64:128, koff + k, :], qr[:, 2 * k + 1, :])

            h1T = ppool.tile([128, 4, bl], F32)
            for m in range(4):
                ps = pspool.tile([128, 512], F32, tag="ps")
                for kt in range(16):
                    wtile = wpool.tile([128, 128], F32, tag="wt")
                    nc.sync.dma_start(wtile[:], wh[kt, :, m, :])
                    nc.tensor.matmul(ps[:, 0:bl], wtile[:], hT[:, kt, :],
                                     start=(kt == 0), stop=(kt == 15))
                leaky_from_psum(h1T[:, m, :], ps[:, 0:bl], bh_sb[:, m:m + 1],
                                [128, bl], "lk_h")

            lps = pspool.tile([128, 512], F32, tag="ps")
            for kt in range(4):
                w2 = wpool.tile([128, 2], F32, tag="w2")
                nc.sync.dma_start(w2[:], wh2[kt])
                nc.tensor.matmul(lps[0:2, 0:bl], w2[:], h1T[:, kt, :],
                                 start=(kt == 0), stop=(kt == 3))
            logits_sb = ppool.tile([2, bl], F32)
            nc.scalar.activation(logits_sb[:], lps[0:2, 0:bl], ACT.Identity,
                                 bias=bh2_sb[:])
            nc.sync.dma_start(out_logits[:], logits_sb[:])

    nc.finalize()
    return nc


# ---------------------------------------------------------------------------
# host-side helpers
# ---------------------------------------------------------------------------

def _fold_bn(g, b, rm, rv):
    s = g / np.sqrt(rv + BN_EPS)
    return s.astype(np.float64), (b - rm * s).astype(np.float64)


def prepare_inputs(d_emb, t_emb, t_mask, params, bl=BL):
    """Returns (shared_map, per_core_fn(core)->dict)."""
    p = params
    ncores = d_emb.shape[0] // bl

    sf, shf = _fold_bn(p["feat_bn_g"], p["feat_bn_b"], p["feat_bn_rm"], p["feat_bn_rv"])
    wfeat = (p["feat_w"].astype(np.float64) * sf[:, None, None])
    bfeat_v = (p["feat_b"] * sf + shf).astype(np.float32)
    wattn = p["attn_w"].astype(np.float64)
    battn_v = p["attn_b"].astype(np.float32)

    def slab(w):
        # (768, 1280, 9) -> (6, 10, 128ci, 9, 128co) fp16
        a = w.reshape(NCO, 128, NCI, 128, KW)
        return np.ascontiguousarray(a.transpose(0, 2, 3, 4, 1)).astype(np.float16)

    wslab = np.stack([slab(wattn), slab(wfeat)], axis=0)

    sd, shd = _fold_bn(p["drug_bn_g"], p["drug_bn_b"], p["drug_bn_rm"], p["drug_bn_rv"])
    Wd = (p["drug_w"].astype(np.float64) * sd[:, None]).T.astype(np.float32)  # (768, 1024)
    bd_v = (p["drug_b"] * sd + shd).astype(np.float32)
    wd_t = np.ascontiguousarray(Wd.reshape(6, 128, 8, 128))

    sp_, shp = _fold_bn(p["proj_bn_g"], p["proj_bn_b"], p["proj_bn_rm"], p["proj_bn_rv"])
    Wp = (p["proj_w"].astype(np.float64) * sp_[:, None]).T.astype(np.float32)  # (1536, 1024)
    bp_v = (p["proj_b"] * sp_ + shp).astype(np.float32)
    wp_t = np.ascontiguousarray(Wp.reshape(12, 128, 8, 128))

    sh_, shh = _fold_bn(p["head_bn_g"], p["head_bn_b"], p["head_bn_rm"], p["head_bn_rv"])
    Wh = (p["head1_w"].astype(np.float64) * sh_[:, None]).T.astype(np.float32)  # (2048, 512)
    bh_v = (p["head1_b"] * sh_ + shh).astype(np.float32)
    wh_t = np.ascontiguousarray(Wh.reshape(16, 128, 4, 128))

    wh2_t = np.ascontiguousarray(p["head2_w"].T.astype(np.float32).reshape(4, 128, 2))
    bh2_v = p["head2_b"].astype(np.float32).reshape(2, 1)

    def cb_pack(cb):
        aug = np.concatenate([-2.0 * cb.T, (cb * cb).sum(1)[None, :]], axis=0)
        return aug.astype(np.float32), np.ascontiguousarray(cb.reshape(4, 128, CD)).astype(np.float32)

    cba_d, cbt_d = cb_pack(p["cb_d"].astype(np.float64))
    cba_t, cbt_t = cb_pack(p["cb_t"].astype(np.float64))

    shared = {
        "wslab": wslab,
        "wd": wd_t, "bd": bd_v.reshape(8, 128),
        "battn": battn_v.reshape(NCO, 128), "bfeat": bfeat_v.reshape(NCO, 128),
        "wp": wp_t, "bp": bp_v.reshape(8, 128),
        "cbaug": np.stack([cba_d, cba_t]), "cbt": np.stack([cbt_d, cbt_t]),
        "eye": np.eye(128, dtype=np.float32),
        "wh": wh_t, "bh": bh_v.reshape(4, 128),
        "wh2": wh2_t, "bh2": bh2_v,
    }

    xt_all = np.zeros((d_emb.shape[0], CI, LP), np.float16)
    xt_all[:, :, 4:4 + L] = t_emb.transpose(0, 2, 1)
    xt_all = xt_all.reshape(d_emb.shape[0], NCI, 128, LP)
    madd_all = np.where(t_mask, np.float32(0.0), np.float32(-1e9))
    dembT_all = d_emb.T.astype(np.float32)  # (768, B)

    def per_core(c):
        rows = slice(c * bl, (c + 1) * bl)
        return {
            "xt": np.ascontiguousarray(xt_all[rows]),
            "madd": np.ascontiguousarray(madd_all[rows]),
            "dembT": np.ascontiguousarray(dembT_all[:, rows].reshape(6, 128, bl)),
            **shared,
        }

    return per_core, ncores


def _digamma(x):
    x = np.asarray(x, np.float64)
    res = np.zeros_like(x)
    y = x.copy()
    for _ in range(12):
        m = y < 12.0
        if not m.any():
            break
        res = res - np.where(m, 1.0 / y, 0.0)
        y = np.where(m, y + 1.0, y)
    inv = 1.0 / y
    inv2 = inv * inv
    res += np.log(y) - 0.5 * inv - inv2 * (
        1.0 / 12 - inv2 * (1.0 / 120 - inv2 * (1.0 / 252 - inv2 * (1.0 / 240))))
    return res


def _gammaln(x):
    x = np.asarray(x, np.float64)
    res = np.zeros_like(x)
    y = x.copy()
    for _ in range(12):
        m = y < 12.0
        if not m.any():
            break
        res = res - np.where(m, np.log(y), 0.0)
        y = np.where(m, y + 1.0, y)
    inv = 1.0 / y
    inv2 = inv * inv
    res += (y - 0.5) * np.log(y) - y + 0.5 * np.log(2.0 * np.pi) + inv * (
        1.0 / 12 - inv2 * (1.0 / 360 - inv2 * (1.0 / 1260)))
    return res


def finalize_outputs(results, y, bl=BL):
    ncores = len(results)
    n_slices = ncores * bl * 16

    def perp(key):
        counts = np.zeros(KCB, np.float64)
        for r in results:
            counts += r[key][0].astype(np.float64)
        avg = (counts / n_slices).astype(np.float32)
        ent = -np.sum((avg * np.log(avg + np.float32(1e-10))).astype(np.float32))
        return np.float32(np.exp(ent))

    d_perp = perp("out_counts_d")
    t_perp = perp("out_counts_t")

    scal = np.zeros(8, np.float64)
    for r in results:
        scal += r["out_scalars"][0].astype(np.float64)
    denom = n_slices * CD
    d_vq = np.float32((scal[0] + scal[1]) / denom)
    t_vq = np.float32((scal[2] + scal[3]) / denom)

    logits = np.concatenate([r["out_logits"].T for r in results], axis=0).astype(np.float64)
    y = np.asarray(y).astype(np.int64)
    y1h = np.eye(N_CLASS, dtype=np.float64)[y]
    alpha = np.log1p(np.exp(-np.abs(logits))) + np.maximum(logits, 0.0) + 1.0
    S = alpha.sum(-1, keepdims=True)
    ce = (y1h * (_digamma(S) - _digamma(alpha))).sum(-1)
    a_t = y1h + (1.0 - y1h) * alpha
    St = a_t.sum(-1, keepdims=True)
    kl = (_gammaln(St[..., 0]) - _gammaln(a_t).sum(-1) - _gammaln(float(N_CLASS))
          + ((a_t - 1.0) * (_digamma(a_t) - _digamma(St))).sum(-1))
    class_loss = np.float32(np.mean(ce + EVD_LAMBDA * kl))

    loss = np.float32(class_loss + d_vq + t_vq)
    return (loss, class_loss, d_vq, t_vq, d_perp, t_perp)


_NC_CACHE = {}


def get_nc(bl=BL):
    if bl not in _NC_CACHE:
        _NC_CACHE[bl] = build_nc(bl)
    return _NC_CACHE[bl]


def kernel(d_emb, t_emb, t_mask, y, params):
    d_emb = np.asarray(d_emb, np.float32)
    t_emb = np.asarray(t_emb, np.float32)
    t_mask = np.asarray(t_mask)
    params = {k: np.asarray(v, np.float32) for k, v in params.items()}

    nc = get_nc(BL)
    per_core, ncores = prepare_inputs(d_emb, t_emb, t_mask, params, BL)
    in_maps = [per_core(c) for c in range(ncores)]
    res = run_bass_kernel_spmd(nc, in_maps, list(range(ncores)))
    return finalize_outputs(res.results, y, BL)


# revision 10
# speedup vs baseline: 6.5938x; 6.5938x over previous
"""Trainium2 Bass kernel for nn_CYPClassifier (vq_codebook).

Data-parallel over batch B=64 across 8 NeuronCores (8 rows/core).
Heavy compute (two 1280->768 k=9 conv1ds as 9-tap shifted matmuls in fp16,
attention pooling, projections, VQ distance/argmin/histogram, evidence head)
runs on device; the host only shards inputs, folds BN constants into
weights, sums per-core partial losses/histograms and evaluates the final
closed-form scalars (digamma/gammaln on 64x2 logits).
"""

import sys

import numpy as np

for _p in ("/opt/trn_rl_repo", "/root/.axon_site/_ro/trn_rl_repo"):
    if _p not in sys.path:
        sys.path.append(_p)

import concourse.bacc as bacc
import concourse.bass as bass
import concourse.tile as tile
from concourse import mybir
from concourse.bass_utils import run_bass_kernel_spmd

F32 = mybir.dt.float32
F16 = mybir.dt.float16
ALU = mybir.AluOpType
ACT = mybir.ActivationFunctionType
AX = mybir.AxisListType

NCORES = 8
B = 64
BL = B // NCORES          # batch rows per core
GN = 4                    # batch rows per PSUM group (8 banks per conv pass)
L = 1024
LP = L + 8                # padded length (4 each side)
CI, CO = 1280, 768        # conv in/out channels
NCI, NCO = CI // 128, CO // 128
KW = 9
KCB, CD = 512, 64         # codebook size, code dim
BN_EPS = 1e-5
EVD_LAMBDA = 0.1
N_CLASS = 2


def _group_ranges(bl, gn=GN):
    out = []
    s = 0
    while s < bl:
        n = min(gn, bl - s)
        out.append((s, n))
        s += n
    return out


def build_nc(bl=BL, reps=1):
    ns = bl * 16
    nc = bacc.Bacc("TRN2", target_bir_lowering=False, debug=False)

    # ---------------- DRAM I/O ----------------
    xt = nc.dram_tensor("xt", [bl, NCI, 128, LP], F16, kind="ExternalInput")
    wslab = nc.dram_tensor("wslab", [2, NCO, NCI, 128, KW, 128], F16, kind="ExternalInput")
    madd = nc.dram_tensor("madd", [bl, L], F32, kind="ExternalInput")
    dembT = nc.dram_tensor("dembT", [6, 128, bl], F32, kind="ExternalInput")
    wd = nc.dram_tensor("wd", [6, 128, 8, 128], F32, kind="ExternalInput")
    bd = nc.dram_tensor("bd", [8, 128], F32, kind="ExternalInput")
    battn = nc.dram_tensor("battn", [NCO, 128], F32, kind="ExternalInput")
    bfeat = nc.dram_tensor("bfeat", [NCO, 128], F32, kind="ExternalInput")
    wp = nc.dram_tensor("wp", [12, 128, 8, 128], F16, kind="ExternalInput")
    bp = nc.dram_tensor("bp", [8, 128], F32, kind="ExternalInput")
    cbaug = nc.dram_tensor("cbaug", [2, 65, KCB], F32, kind="ExternalInput")
    cbt = nc.dram_tensor("cbt", [2, 4, 128, CD], F32, kind="ExternalInput")
    eye = nc.dram_tensor("eye", [128, 128], F32, kind="ExternalInput")
    wh = nc.dram_tensor("wh", [16, 128, 4, 128], F16, kind="ExternalInput")
    bh = nc.dram_tensor("bh", [4, 128], F32, kind="ExternalInput")
    wh2 = nc.dram_tensor("wh2", [4, 128, 2], F16, kind="ExternalInput")
    bh2 = nc.dram_tensor("bh2", [2, 1], F32, kind="ExternalInput")

    out_counts_d = nc.dram_tensor("out_counts_d", [1, KCB], F32, kind="ExternalOutput")
    out_counts_t = nc.dram_tensor("out_counts_t", [1, KCB], F32, kind="ExternalOutput")
    out_scalars = nc.dram_tensor("out_scalars", [1, 8], F32, kind="ExternalOutput")
    out_logits = nc.dram_tensor("out_logits", [2, bl], F32, kind="ExternalOutput")

    groups = _group_ranges(bl)

    with tile.TileContext(nc) as tc:
        with (
            tc.tile_pool(name="const", bufs=1) as cpool,
            tc.tile_pool(name="persist", bufs=1) as ppool,
            tc.tile_pool(name="xp", bufs=1) as xpool,
            tc.tile_pool(name="slab", bufs=2) as slabpool,
            tc.tile_pool(name="maddp", bufs=1) as maddpool,
            tc.tile_pool(name="work", bufs=2) as workpool,
            tc.tile_pool(name="exp", bufs=1) as expool,
            tc.tile_pool(name="scr", bufs=1) as scrpool,
            tc.tile_pool(name="small", bufs=4) as smpool,
            tc.tile_pool(name="wt", bufs=8) as wpool,
            tc.tile_pool(name="vq", bufs=1) as vqpool,
            tc.tile_pool(name="ps", bufs=8, space="PSUM") as pspool,
        ):
            # ---------------- constants ----------------
            ones = cpool.tile([128, 1], F32)
            nc.vector.memset(ones[:], 1.0)
            eye_sb = cpool.tile([128, 128], F32)
            nc.sync.dma_start(eye_sb[:], eye[:])

            def col_tile(dram, ncols):
                t = cpool.tile([128, ncols], F32, tag=dram.name + "_sb", name=dram.name + "_sb")
                nc.sync.dma_start(t[:], dram[:].rearrange("m p -> p m"))
                return t

            bd_sb = col_tile(bd, 8)
            battn_sb = col_tile(battn, NCO)
            bfeat_sb = col_tile(bfeat, NCO)
            bp_sb = col_tile(bp, 8)
            bh_sb = col_tile(bh, 4)
            bh2_sb = cpool.tile([2, 1], F32)
            nc.sync.dma_start(bh2_sb[:], bh2[:])

            dembT_sb = cpool.tile([128, 6, bl], F32)
            nc.sync.dma_start(dembT_sb[:], dembT[:].rearrange("k p b -> p k b"))

            def leaky_from_psum(out_ap, ps_ap, bias_ap, tmp_shape, tag):
                tmp = smpool.tile(tmp_shape, F32, tag=tag, name=tag)
                nc.scalar.activation(tmp[:], ps_ap, ACT.Identity, bias=bias_ap)
                nc.vector.scalar_tensor_tensor(out_ap, in0=tmp[:], scalar=0.01,
                                               in1=tmp[:], op0=ALU.mult, op1=ALU.max)

            def run_vq(zT, cb_idx, out_counts, scal_sb, scal_off, qT):
                # z_aug: [65, ns]; column n = b*16 + s holds slice s of row b
                zaug = vqpool.tile([65, ns], F32, tag="zaug", name="zaug")
                nc.vector.memset(zaug[64:65, :], 1.0)
                zr = zaug[0:64, :].rearrange("p (b s) -> p s b", s=16)
                for m in range(8):
                    nc.vector.tensor_copy(zr[:, 2 * m, :], zT[0:64, m, :])
                    # partitions 64-127 -> 0-63 requires a DMA
                    nc.sync.dma_start(zr[:, 2 * m + 1, :], zT[64:128, m, :])

                cba = vqpool.tile([65, KCB], F32, tag="cba", name="cba")
                nc.sync.dma_start(cba[:], cbaug[cb_idx])
                eps = pspool.tile([128, 512], F32, tag="ps", name="eps")
                nc.tensor.matmul(eps[0:ns, :], zaug[:], cba[:], start=True, stop=True)
                e_sb = vqpool.tile([ns, KCB], F32, tag="esb", name="esb")
                nc.vector.tensor_copy(e_sb[:], eps[0:ns, :])
                mmin = smpool.tile([ns, 1], F32, tag="mmin", name="mmin")
                nc.vector.tensor_reduce(mmin[:], e_sb[:], axis=AX.X, op=ALU.min)
                onehot = vqpool.tile([ns, KCB], F32, tag="onehot", name="onehot")
                nc.vector.tensor_single_scalar(onehot[:], e_sb[:], mmin[:], ALU.is_equal)

                # histogram: counts = ones^T @ onehot
                cps = pspool.tile([128, 512], F32, tag="ps", name="cps")
                nc.tensor.matmul(cps[0:1, :], ones[0:ns, :], onehot[:], start=True, stop=True)
                counts_sb = vqpool.tile([1, KCB], F32, tag="counts_sb", name="counts_sb")
                nc.vector.tensor_copy(counts_sb[:], cps[0:1, :])
                nc.sync.dma_start(out_counts[:], counts_sb[:])

                # vq loss partials: sum(|z|^2) and sum(min e)
                sps = pspool.tile([128, 512], F32, tag="ps", name="sps")
                nc.tensor.matmul(sps[0:1, 0:1], mmin[:], ones[0:ns, :], start=True, stop=True)
                nc.scalar.copy(scal_sb[:, scal_off + 1:scal_off + 2], sps[0:1, 0:1])
                zsq = vqpool.tile([64, ns], F32, tag="zsq", name="zsq")
                nc.vector.tensor_mul(zsq[:], zaug[0:64, :], zaug[0:64, :])
                zrow = smpool.tile([64, 1], F32, tag="zrow", name="zrow")
                nc.vector.reduce_sum(zrow[:], zsq[:], axis=AX.X)
                z2ps = pspool.tile([128, 512], F32, tag="ps", name="z2ps")
                nc.tensor.matmul(z2ps[0:1, 0:1], zrow[:], ones[0:64, :], start=True, stop=True)
                nc.scalar.copy(scal_sb[:, scal_off:scal_off + 1], z2ps[0:1, 0:1])

                # q = onehot @ cb via PE transpose of onehot (fp16 out for head)
                qps = pspool.tile([128, 512], F32, tag="ps", name="qps")
                for j in range(4):
                    tps = pspool.tile([128, 512], F32, tag="ps", name="tps")
                    nc.tensor.transpose(tps[0:128, 0:ns], onehot[:, j * 128:(j + 1) * 128],
                                        eye_sb[0:ns, 0:ns])
                    ohT = vqpool.tile([128, ns], F32, tag="ohT", name="ohT")
                    nc.vector.tensor_copy(ohT[:], tps[0:128, 0:ns])
                    cb_sb = vqpool.tile([128, CD], F32, tag="cbsb", name="cbsb")
                    nc.sync.dma_start(cb_sb[:], cbt[cb_idx, j])
                    nc.tensor.matmul(qps[0:64, 0:ns], cb_sb[:], ohT[:],
                                     start=(j == 0), stop=(j == 3))
                nc.vector.tensor_copy(qT[:], qps[0:64, 0:ns])

            def _body():
                # ---------------- drug encoder ----------------
                dzT = ppool.tile([128, 8, bl], F32, name="dzT")
                for m in range(8):
                    ps = pspool.tile([128, 512], F32, tag="ps", name="ps_drug")
                    for kt in range(6):
                        wtile = wpool.tile([128, 128], F32, tag="wt32", name="wt_d")
                        nc.sync.dma_start(wtile[:], wd[kt, :, m, :])
                        nc.tensor.matmul(ps[:, 0:bl], wtile[:], dembT_sb[:, kt, :],
                                         start=(kt == 0), stop=(kt == 5))
                    leaky_from_psum(dzT[:, m, :], ps[:, 0:bl], bd_sb[:, m:m + 1],
                                    [128, bl], "lk_d")

                scal_sb = ppool.tile([1, 8], F32, name="scal_sb")
                nc.vector.memset(scal_sb[:], 0.0)
                qTd = ppool.tile([64, ns], F16, name="qTd")
                run_vq(dzT, 0, out_counts_d, scal_sb, 0, qTd)

                # ---------------- target conv encoder ----------------
                combT = ppool.tile([128, 12, bl], F32, name="combT")

                for g0, gn in groups:
                    xs = {}
                    madd_sb = {}
                    for bi in range(gn):
                        b = g0 + bi
                        for ci in range(NCI):
                            t = xpool.tile([128, LP], F16, tag=f"x_{bi}_{ci}", name=f"x_{bi}_{ci}")
                            nc.sync.dma_start(t[:], xt[b, ci])
                            xs[bi, ci] = t
                        mt = maddpool.tile([128, L], F32, tag=f"madd_{bi}", name=f"madd_{bi}")
                        src = madd[b, :]
                        nc.sync.dma_start(
                            mt[:],
                            bass.AP(tensor=src.tensor, offset=src.offset,
                                    ap=[[0, 128]] + list(src.ap)),
                        )
                        madd_sb[bi] = mt

                    for co in range(NCO):
                        exs = {}
                        rsums = {}
                        for conv_idx in (0, 1):  # 0 = attn, 1 = feat
                            pst = {}
                            for bi in range(gn):
                                for li in range(2):
                                    pst[bi, li] = pspool.tile([128, 512], F32, tag="ps",
                                                              name=f"pst_{bi}_{li}")
                            slab = slabpool.tile([128, NCI, KW, 128], F16, tag="slab",
                                                 name="slab")
                            nc.sync.dma_start(
                                slab[:], wslab[conv_idx, co].rearrange("c p k j -> p c k j"))
                            for ci in range(NCI):
                                for k in range(KW):
                                    lhsT = slab[:, ci, k, :]
                                    for bi in range(gn):
                                        for li in range(2):
                                            rhs = xs[bi, ci][:, li * 512 + k: li * 512 + k + 512]
                                            nc.tensor.matmul(
                                                pst[bi, li][:], lhsT, rhs,
                                                start=(ci == 0 and k == 0),
                                                stop=(ci == NCI - 1 and k == KW - 1))
                            if conv_idx == 0:
                                # attn: softmax over L with additive mask
                                for bi in range(gn):
                                    am = workpool.tile([128, L], F32, tag="Am", name="am")
                                    for li in range(2):
                                        nc.vector.scalar_tensor_tensor(
                                            am[:, li * 512:(li + 1) * 512],
                                            in0=pst[bi, li][:],
                                            scalar=battn_sb[:, co:co + 1],
                                            in1=madd_sb[bi][:, li * 512:(li + 1) * 512],
                                            op0=ALU.add, op1=ALU.add)
                                    rmax = smpool.tile([128, 1], F32, tag="rmax", name="rmax")
                                    nc.vector.tensor_reduce(rmax[:], am[:], axis=AX.X, op=ALU.max)
                                    nrmax = smpool.tile([128, 1], F32, tag="nrmax", name="nrmax")
                                    nc.vector.tensor_scalar_mul(nrmax[:], rmax[:], -1.0)
                                    ex = expool.tile([128, L], F32, tag=f"ex_{bi}", name=f"ex_{bi}")
                                    sumex = smpool.tile([128, 1], F32, tag="sumex", name="sumex")
                                    nc.scalar.activation(ex[:], am[:], ACT.Exp,
                                                         bias=nrmax[:], scale=1.0,
                                                         accum_out=sumex[:])
                                    rsum = smpool.tile([128, 1], F32, tag=f"rsum_{bi}",
                                                       name=f"rsum_{bi}")
                                    nc.vector.reciprocal(rsum[:], sumex[:])
                                    exs[bi] = ex
                                    rsums[bi] = rsum
                            else:
                                # feat: weighted sum with softmax weights + masked gmax
                                for bi in range(gn):
                                    b = g0 + bi
                                    ws0 = smpool.tile([128, 1], F32, tag="ws0", name="ws0")
                                    ws1 = smpool.tile([128, 1], F32, tag="ws1", name="ws1")
                                    for li, wsx in ((0, ws0), (1, ws1)):
                                        sc = scrpool.tile([128, 512], F32, tag="sc", name="sc")
                                        nc.vector.scalar_tensor_tensor(
                                            sc[:],
                                            in0=pst[bi, li][:],
                                            scalar=bfeat_sb[:, co:co + 1],
                                            in1=exs[bi][:, li * 512:(li + 1) * 512],
                                            op0=ALU.add, op1=ALU.mult,
                                            accum_out=wsx[:])
                                    wsum = smpool.tile([128, 1], F32, tag="wsum", name="wsum")
                                    nc.vector.tensor_add(wsum[:], ws0[:], ws1[:])
                                    nc.vector.tensor_scalar_mul(
                                        combT[:, co, b:b + 1], wsum[:], rsums[bi][:])
                                    gm0 = smpool.tile([128, 1], F32, tag="gm0", name="gm0")
                                    gm1 = smpool.tile([128, 1], F32, tag="gm1", name="gm1")
                                    for li, gmx in ((0, gm0), (1, gm1)):
                                        sc2 = scrpool.tile([128, 512], F32, tag="sc2", name="sc2")
                                        nc.vector.tensor_add(
                                            sc2[:], pst[bi, li][:],
                                            madd_sb[bi][:, li * 512:(li + 1) * 512])
                                        nc.vector.tensor_reduce(gmx[:], sc2[:], axis=AX.X,
                                                                op=ALU.max)
                                    gmc = smpool.tile([128, 1], F32, tag="gmc", name="gmc")
                                    nc.vector.tensor_max(gmc[:], gm0[:], gm1[:])
                                    nc.scalar.activation(
                                        combT[:, 6 + co, b:b + 1], gmc[:], ACT.Identity,
                                        bias=bfeat_sb[:, co:co + 1])

                # ---------------- proj ----------------
                combT_h = ppool.tile([128, 12, bl], F16, name="combT_h")
                nc.vector.tensor_copy(combT_h[:], combT[:])
                tzT = ppool.tile([128, 8, bl], F32, name="tzT")
                for m in range(8):
                    ps = pspool.tile([128, 512], F32, tag="ps", name="ps_proj")
                    for kt in range(12):
                        wtile = wpool.tile([128, 128], F16, tag="wt", name="wt_p")
                        nc.sync.dma_start(wtile[:], wp[kt, :, m, :])
                        nc.tensor.matmul(ps[:, 0:bl], wtile[:], combT_h[:, kt, :],
                                         start=(kt == 0), stop=(kt == 11))
                    leaky_from_psum(tzT[:, m, :], ps[:, 0:bl], bp_sb[:, m:m + 1],
                                    [128, bl], "lk_p")

                qTt = ppool.tile([64, ns], F16, name="qTt")
                run_vq(tzT, 1, out_counts_t, scal_sb, 2, qTt)
                nc.sync.dma_start(out_scalars[:], scal_sb[:])

                # ---------------- evidence head ----------------
                hT = ppool.tile([128, 16, bl], F16, name="hT")
                for src_q, koff in ((qTd, 0), (qTt, 8)):
                    qr = src_q[:].rearrange("p (b s) -> p s b", s=16)
                    for k in range(8):
                        nc.vector.tensor_copy(hT[0:64, koff + k, :], qr[:, 2 * k, :])
                        nc.sync.dma_start(hT[64:128, koff + k, :], qr[:, 2 * k + 1, :])

                h1T = ppool.tile([128, 4, bl], F16, name="h1T")
                for m in range(4):
                    ps = pspool.tile([128, 512], F32, tag="ps", name="ps_head")
                    for kt in range(16):
                        wtile = wpool.tile([128, 128], F16, tag="wt", name="wt_h")
                        nc.sync.dma_start(wtile[:], wh[kt, :, m, :])
                        nc.tensor.matmul(ps[:, 0:bl], wtile[:], hT[:, kt, :],
                                         start=(kt == 0), stop=(kt == 15))
                    leaky_from_psum(h1T[:, m, :], ps[:, 0:bl], bh_sb[:, m:m + 1],
                                    [128, bl], "lk_h")

                lps = pspool.tile([128, 512], F32, tag="ps", name="lps")
                for kt in range(4):
                    w2 = wpool.tile([128, 2], F16, tag="w2", name="w2")
                    nc.sync.dma_start(w2[:], wh2[kt])
                    nc.tensor.matmul(lps[0:2, 0:bl], w2[:], h1T[:, kt, :],
                                     start=(kt == 0), stop=(kt == 3))
                logits_sb = ppool.tile([2, bl], F32, name="logits_sb")
                nc.scalar.activation(logits_sb[:], lps[0:2, 0:bl], ACT.Identity,
                                     bias=bh2_sb[:])
                nc.sync.dma_start(out_logits[:], logits_sb[:])

            for _rep in range(reps):
                _body()

    nc.finalize()
    return nc


# ---------------------------------------------------------------------------
# host-side helpers
# ---------------------------------------------------------------------------

def _fold_bn(g, b, rm, rv):
    s = g / np.sqrt(rv + BN_EPS)
    return s.astype(np.float64), (b - rm * s).astype(np.float64)


def prepare_inputs(d_emb, t_emb, t_mask, params, bl=BL):
    """Returns (per_core_fn(core)->dict, ncores)."""
    p = params
    ncores = d_emb.shape[0] // bl

    sf, shf = _fold_bn(p["feat_bn_g"], p["feat_bn_b"], p["feat_bn_rm"], p["feat_bn_rv"])
    wfeat = (p["feat_w"].astype(np.float64) * sf[:, None, None])
    bfeat_v = (p["feat_b"] * sf + shf).astype(np.float32)
    wattn = p["attn_w"].astype(np.float64)
    battn_v = p["attn_b"].astype(np.float32)

    def slab(w):
        # (768, 1280, 9) -> (6, 10, 128ci, 9, 128co) fp16
        a = w.reshape(NCO, 128, NCI, 128, KW)
        return np.ascontiguousarray(a.transpose(0, 2, 3, 4, 1)).astype(np.float16)

    wslab = np.stack([slab(wattn), slab(wfeat)], axis=0)

    sd, shd = _fold_bn(p["drug_bn_g"], p["drug_bn_b"], p["drug_bn_rm"], p["drug_bn_rv"])
    Wd = (p["drug_w"].astype(np.float64) * sd[:, None]).T.astype(np.float32)  # (768, 1024)
    bd_v = (p["drug_b"] * sd + shd).astype(np.float32)
    wd_t = np.ascontiguousarray(Wd.reshape(6, 128, 8, 128))

    sp_, shp = _fold_bn(p["proj_bn_g"], p["proj_bn_b"], p["proj_bn_rm"], p["proj_bn_rv"])
    Wp = (p["proj_w"].astype(np.float64) * sp_[:, None]).T.astype(np.float16)  # (1536, 1024)
    bp_v = (p["proj_b"] * sp_ + shp).astype(np.float32)
    wp_t = np.ascontiguousarray(Wp.reshape(12, 128, 8, 128))

    sh_, shh = _fold_bn(p["head_bn_g"], p["head_bn_b"], p["head_bn_rm"], p["head_bn_rv"])
    Wh = (p["head1_w"].astype(np.float64) * sh_[:, None]).T.astype(np.float16)  # (2048, 512)
    bh_v = (p["head1_b"] * sh_ + shh).astype(np.float32)
    wh_t = np.ascontiguousarray(Wh.reshape(16, 128, 4, 128))

    wh2_t = np.ascontiguousarray(p["head2_w"].T.astype(np.float16).reshape(4, 128, 2))
    bh2_v = p["head2_b"].astype(np.float32).reshape(2, 1)

    def cb_pack(cb):
        aug = np.concatenate([-2.0 * cb.T, (cb * cb).sum(1)[None, :]], axis=0)
        return aug.astype(np.float32), np.ascontiguousarray(cb.reshape(4, 128, CD)).astype(np.float32)

    cba_d, cbt_d = cb_pack(p["cb_d"].astype(np.float64))
    cba_t, cbt_t = cb_pack(p["cb_t"].astype(np.float64))

    shared = {
        "wslab": wslab,
        "wd": wd_t, "bd": bd_v.reshape(8, 128),
        "battn": battn_v.reshape(NCO, 128), "bfeat": bfeat_v.reshape(NCO, 128),
        "wp": wp_t, "bp": bp_v.reshape(8, 128),
        "cbaug": np.stack([cba_d, cba_t]), "cbt": np.stack([cbt_d, cbt_t]),
        "eye": np.eye(128, dtype=np.float32),
        "wh": wh_t, "bh": bh_v.reshape(4, 128),
        "wh2": wh2_t, "bh2": bh2_v,
    }

    xt_all = np.zeros((d_emb.shape[0], CI, LP), np.float16)
    xt_all[:, :, 4:4 + L] = t_emb.transpose(0, 2, 1)
    xt_all = xt_all.reshape(d_emb.shape[0], NCI, 128, LP)
    madd_all = np.where(t_mask, np.float32(0.0), np.float32(-1e9))
    dembT_all = d_emb.T.astype(np.float32)  # (768, B)

    def per_core(c):
        rows = slice(c * bl, (c + 1) * bl)
        return {
            "xt": np.ascontiguousarray(xt_all[rows]),
            "madd": np.ascontiguousarray(madd_all[rows]),
            "dembT": np.ascontiguousarray(dembT_all[:, rows].reshape(6, 128, bl)),
            **shared,
        }

    return per_core, ncores


def _digamma(x):
    x = np.asarray(x, np.float64)
    res = np.zeros_like(x)
    y = x.copy()
    for _ in range(12):
        m = y < 12.0
        if not m.any():
            break
        res = res - np.where(m, 1.0 / y, 0.0)
        y = np.where(m, y + 1.0, y)
    inv = 1.0 / y
    inv2 = inv * inv
    res += np.log(y) - 0.5 * inv - inv2 * (
        1.0 / 12 - inv2 * (1.0 / 120 - inv2 * (1.0 / 252 - inv2 * (1.0 / 240))))
    return res


def _gammaln(x):
    x = np.asarray(x, np.float64)
    res = np.zeros_like(x)
    y = x.copy()
    for _ in range(12):
        m = y < 12.0
        if not m.any():
            break
        res = res - np.where(m, np.log(y), 0.0)
        y = np.where(m, y + 1.0, y)
    inv = 1.0 / y
    inv2 = inv * inv
    res += (y - 0.5) * np.log(y) - y + 0.5 * np.log(2.0 * np.pi) + inv * (
        1.0 / 12 - inv2 * (1.0 / 360 - inv2 * (1.0 / 1260)))
    return res


def finalize_outputs(results, y, bl=BL):
    ncores = len(results)
    n_slices = ncores * bl * 16

    def perp(key):
        counts = np.zeros(KCB, np.float64)
        for r in results:
            counts += r[key][0].astype(np.float64)
        avg = (counts / n_slices).astype(np.float32)
        ent = -np.sum((avg * np.log(avg + np.float32(1e-10))).astype(np.float32))
        return np.float32(np.exp(ent))

    d_perp = perp("out_counts_d")
    t_perp = perp("out_counts_t")

    scal = np.zeros(8, np.float64)
    for r in results:
        scal += r["out_scalars"][0].astype(np.float64)
    denom = n_slices * CD
    d_vq = np.float32((scal[0] + scal[1]) / denom)
    t_vq = np.float32((scal[2] + scal[3]) / denom)

    logits = np.concatenate([r["out_logits"].T for r in results], axis=0).astype(np.float64)
    y = np.asarray(y).astype(np.int64)
    y1h = np.eye(N_CLASS, dtype=np.float64)[y]
    alpha = np.log1p(np.exp(-np.abs(logits))) + np.maximum(logits, 0.0) + 1.0
    S = alpha.sum(-1, keepdims=True)
    ce = (y1h * (_digamma(S) - _digamma(alpha))).sum(-1)
    a_t = y1h + (1.0 - y1h) * alpha
    St = a_t.sum(-1, keepdims=True)
    kl = (_gammaln(St[..., 0]) - _gammaln(a_t).sum(-1) - _gammaln(float(N_CLASS))
          + ((a_t - 1.0) * (_digamma(a_t) - _digamma(St))).sum(-1))
    class_loss = np.float32(np.mean(ce + EVD_LAMBDA * kl))

    loss = np.float32(class_loss + d_vq + t_vq)
    return (loss, class_loss, d_vq, t_vq, d_perp, t_perp)


_NC_CACHE = {}


def get_nc(bl=BL):
    if bl not in _NC_CACHE:
        _NC_CACHE[bl] = build_nc(bl)
    return _NC_CACHE[bl]


def kernel(d_emb, t_emb, t_mask, y, params):
    d_emb = np.asarray(d_emb, np.float32)
    t_emb = np.asarray(t_emb, np.float32)
    t_mask = np.asarray(t_mask)
    params = {k: np.asarray(v, np.float32) for k, v in params.items()}

    nc = get_nc(BL)
    per_core, ncores = prepare_inputs(d_emb, t_emb, t_mask, params, BL)
    in_maps = [per_core(c) for c in range(ncores)]
    res = run_bass_kernel_spmd(nc, in_maps, list(range(ncores)))
    return finalize_outputs(res.results, y, BL)
